# revision 7
# baseline (speedup 1.0000x reference)
"""Distributed GPT-2 causal attention block for 8 TRN2 NeuronCores.

Sharding: data-parallel over batch (B=2) x tensor-parallel over heads
(16 heads -> 4 groups of 4). core = b*4 + g handles batch b, heads 4g..4g+3.

Per-core kernel (all compute in bf16, f32 PSUM accumulation):
  qT/kT = W[q|k]^T x^T        [2 tiles of 128 = 2 heads each, layout (h d) x S]
  v     = x W_v               [S x (4 heads x 65)], col 64 of each head = ones
  sT    = kT^T qT (transposed scores, row-packed 2 heads/matmul via tile rows)
  PT    = exp(sT/8) (ScalarE), causal: diag block masked multiplicatively
          on GpSimd, blocks above diag never computed
  av    = v_aug^T PT accumulated over key blocks -> [65 x S] PSUM;
          rows 0:64 unnormalized attn out^T, row 64 = softmax denominators r
  rinv  = 1/r on the denominator row (DVE), broadcast to 64 partitions via
          GpSimd partition_broadcast (no DRAM roundtrip)
  attT  = av[0:64] * rinv     [(h d) x S]
  outT  = wp^T attT           [NX x S] partial (sum over head groups on host)

Inputs stream in fine-grained chunks ordered so the first q/k matmuls start
~2us in; q and k accumulation waves for pair 0 interleave so scores can
begin as soon as the first half of qt/kt exists.

Host: shard/cast inputs, run SPMD on cores 0-7, transpose+reduce partials,
fold b_attn's v-bias and b_proj in on the host (exact: softmax rows sum to 1).
"""

import numpy as np
import ml_dtypes

B, S, NX = 2, 2048, 1024
H, D = 16, 64
HPC = 4        # heads per core
KCH = NX // 128  # 8 contraction chunks
SQT = S // 128   # 16 query tiles
SCALE = 0.125    # 1/sqrt(D)

_nc_cache = None


def _sub512(lo, hi):
    """split [lo,hi) on the 512 grid."""
    out = []
    s = lo
    while s < hi:
        e = min((s // 512 + 1) * 512, hi)
        out.append((s, e))
        s = e
    return out


def _emit(nc, tc, bass, mybir, tens):
    dt = mybir.dt
    F32, BF16 = dt.float32, dt.bfloat16
    MULT = mybir.AluOpType.mult
    EXP = mybir.ActivationFunctionType.Exp
    xT, wqk, wv, wp, bq, bk, maskT, outT = tens

    import contextlib
    with contextlib.ExitStack() as ctx:
        consts = ctx.enter_context(tc.tile_pool(name="consts", bufs=1))
        wpool = ctx.enter_context(tc.tile_pool(name="wpool", bufs=1))
        xpool = ctx.enter_context(tc.tile_pool(name="xpool", bufs=1))
        qkpool = ctx.enter_context(tc.tile_pool(name="qkpool", bufs=1))
        vpool = ctx.enter_context(tc.tile_pool(name="vpool", bufs=1))
        ptpool = ctx.enter_context(tc.tile_pool(name="ptpool", bufs=1))
        atpool = ctx.enter_context(tc.tile_pool(name="atpool", bufs=1))
        rpool = ctx.enter_context(tc.tile_pool(name="rpool", bufs=2))
        rbpool = ctx.enter_context(tc.tile_pool(name="rbpool", bufs=2))
        opool = ctx.enter_context(tc.tile_pool(name="opool", bufs=3))
        psp = ctx.enter_context(tc.tile_pool(name="psp", bufs=3, space="PSUM"))
        avp = ctx.enter_context(tc.tile_pool(name="avp", bufs=2, space="PSUM"))

        # ---- SBUF tiles ----
        wqk_sb = wpool.tile([128, KCH, 2 * HPC * D], BF16, tag="wqk")
        wq_sb = wqk_sb.rearrange("p k (w n) -> p k w n", w=2)[:, :, 0, :]
        wk_sb = wqk_sb.rearrange("p k (w n) -> p k w n", w=2)[:, :, 1, :]
        xc_sb = [xpool.tile([128, 2, S], BF16, tag=f"x{c}", name=f"x{c}")
                 for c in range(4)]
        x_sb = [xc_sb[k // 2][:, k % 2, :] for k in range(KCH)]
        bq_sb = consts.tile([128, 2], F32, tag="bq")
        bk_sb = consts.tile([128, 2], F32, tag="bk")
        mask_sb = consts.tile([128, 128], BF16, tag="mask")
        wv_sb = wpool.tile([128, KCH, HPC * D], BF16, tag="wv")
        wp_sb = wpool.tile([128, 2, NX], BF16, tag="wp")

        qt_sb, kt_sb = [], []
        for t in range(2):
            qt_sb.append(qkpool.tile([128, S], BF16, tag=f"qt{t}", name=f"qt{t}"))
            kt_sb.append(qkpool.tile([128, S], BF16, tag=f"kt{t}", name=f"kt{t}"))
        v_sb = [vpool.tile([128, HPC * 65], BF16, tag=f"v{j}", name=f"v{j}")
                for j in range(SQT)]
        attT = [atpool.tile([128, S], BF16, tag=f"attT{hp}", name=f"attT{hp}")
                for hp in range(2)]

        # ---- input DMAs: fine-grained, ordered for the compute ramp ----
        def dma_wqk(k):
            nc.sync.dma_start(out=wqk_sb[:, k, :],
                              in_=wqk.ap()[k * 128:(k + 1) * 128, :])

        def dma_x(c, s):
            nc.sync.dma_start(
                out=xc_sb[c][:, :, s * 512:(s + 1) * 512],
                in_=xT.ap()[c * 256:(c + 1) * 256,
                            s * 512:(s + 1) * 512].rearrange(
                    "(two p) s -> p two s", p=128))

        dma_wqk(0)
        nc.sync.dma_start(out=bq_sb[:, :],
                          in_=bq.ap().rearrange("(t p) o -> p (t o)", p=128))
        nc.sync.dma_start(out=bk_sb[:, :],
                          in_=bk.ap().rearrange("(t p) o -> p (t o)", p=128))
        nc.sync.dma_start(out=mask_sb[:, :], in_=maskT.ap())
        dma_wqk(1)
        dma_x(0, 0)
        dma_x(0, 1)
        for c in (1, 2, 3):
            dma_wqk(2 * c)
            dma_wqk(2 * c + 1)
            dma_x(c, 0)
            dma_x(c, 1)
        for c in range(4):
            dma_x(c, 2)
            dma_x(c, 3)
        nc.sync.dma_start(out=wv_sb[:],
                          in_=wv.ap().rearrange("(k p) n -> p k n", p=128))
        nc.sync.dma_start(out=wp_sb[:],
                          in_=wp.ap().rearrange("(k p) n -> p k n", p=128))

        # ---- q/k projection waves (q and k interleaved per k-chunk) ----
        def emit_qk_tile_steps(t, cpair):
            psq = psp.tile([128, 1024], F32, tag="sps", name=f"qw{t}{cpair[0]}")
            psk = psp.tile([128, 1024], F32, tag="sps", name=f"kw{t}{cpair[0]}")

            def kstep(k):
                for ci, c in enumerate(cpair):
                    nc.tensor.matmul(
                        psq[:, ci * 512:(ci + 1) * 512],
                        lhsT=wq_sb[:, k, t * 128:(t + 1) * 128],
                        rhs=x_sb[k][:, c * 512:(c + 1) * 512],
                        start=(k == 0), stop=(k == KCH - 1))
                for ci, c in enumerate(cpair):
                    nc.tensor.matmul(
                        psk[:, ci * 512:(ci + 1) * 512],
                        lhsT=wk_sb[:, k, t * 128:(t + 1) * 128],
                        rhs=x_sb[k][:, c * 512:(c + 1) * 512],
                        start=(k == 0), stop=(k == KCH - 1))

            def drain():
                for ci, c in enumerate(cpair):
                    nc.vector.tensor_scalar_add(
                        out=qt_sb[t][:, c * 512:(c + 1) * 512],
                        in0=psq[:, ci * 512:(ci + 1) * 512],
                        scalar1=bq_sb[:, t:t + 1])
                for ci, c in enumerate(cpair):
                    nc.vector.tensor_scalar_add(
                        out=kt_sb[t][:, c * 512:(c + 1) * 512],
                        in0=psk[:, ci * 512:(ci + 1) * 512],
                        scalar1=bk_sb[:, t:t + 1])

            steps = [lambda k=k: kstep(k) for k in range(KCH)]
            steps.append(drain)
            return steps

        def emit_v_unit(j):
            vt3 = v_sb[j].rearrange("p (h e) -> p h e", e=65)
            nc.gpsimd.memset(vt3[:, :, 64:65], 1.0)
            ps = psp.tile([128, 512], F32, tag="sps")
            for k in range(KCH):
                nc.tensor.matmul(
                    ps[:, 0:HPC * D],
                    lhsT=x_sb[k][:, j * 128:(j + 1) * 128],
                    rhs=wv_sb[:, k, :],
                    start=(k == 0), stop=(k == KCH - 1))
            nc.vector.tensor_copy(
                out=vt3[:, :, 0:64],
                in_=ps[:, 0:HPC * D].rearrange("p (h d) -> p h d", d=64))

        # PT tiles, keyed (p, hl, j, half); tags shared across pairs
        pt = {}

        def emit_scores_h(p, j, half, lo, hi, hl):
            """one head of pair p for key-block j, sq [lo,hi)."""
            w = hi - lo
            pt_t = ptpool.tile([128, w], BF16, tag=f"pt{hl}_{j}_{half}",
                               name=f"pt{hl}_{j}_{half}_p{p}")
            pt[(p, hl, j, half)] = pt_t
            ps = psp.tile([128, 1024], F32, tag="sps")
            for (a, b) in _sub512(0, w):
                nc.tensor.matmul(
                    ps[:, a:b],
                    lhsT=kt_sb[p][hl * 64:(hl + 1) * 64,
                                  j * 128:(j + 1) * 128],
                    rhs=qt_sb[p][hl * 64:(hl + 1) * 64, lo + a:lo + b],
                    start=True, stop=True)
            nc.scalar.activation(
                out=pt_t[:, :], in_=ps[:, :w], func=EXP, scale=SCALE)
            if lo <= j * 128 < hi:
                o = j * 128 - lo
                # pair-0 masks ride the idle GpSimd; pair-1 masks are woven
                # into the av phase where GpSimd runs the tail broadcasts
                # (FIFO head-of-line: a queued mask would block them)
                eng = nc.gpsimd if p == 0 else nc.vector
                eng.tensor_tensor(
                    out=pt_t[:, o:o + 128], in0=pt_t[:, o:o + 128],
                    in1=mask_sb[:, :], op=MULT)

        def emit_av(p, hl, base, width, j, av):
            h = p * 2 + hl
            half = 0 if base < 1024 else 1
            plo = max(j * 128, half * 1024)   # PT tile column origin
            lo = max(j * 128, base)
            for (a, b) in _sub512(lo, base + width):
                nc.tensor.matmul(
                    av[:, a - base:b - base],
                    lhsT=v_sb[j][:, h * 65:(h + 1) * 65],
                    rhs=pt[(p, hl, j, half)][:, a - plo:b - plo],
                    start=(j == 0), stop=(j == (base + width) // 128 - 1),
                    skip_group_check=True)

        def emit_tail(p, hl, base, width, av):
            h = p * 2 + hl
            # plain copy does the p64 -> p0 quadrant move; recip stays aligned
            rc = rpool.tile([1, 512], F32, tag="rc", name=f"rc{h}_{base}")
            nc.vector.tensor_copy(out=rc[0:1, :width], in_=av[64:65, :width])
            rv = rpool.tile([1, 512], F32, tag="rv", name=f"rv{h}_{base}")
            nc.vector.reciprocal_approx_fast(out=rv[0:1, :width],
                                             in_=rc[0:1, :width])
            rb = rbpool.tile([64, 512], F32, tag="rb", name=f"rb{h}_{base}")
            nc.gpsimd.partition_broadcast(rb[:, :width], rv[0:1, :width],
                                          channels=64)
            gs = slice(base, base + width)
            # 64-ch DVE write routes to either partition half directly
            nc.vector.tensor_tensor(
                out=attT[p][hl * 64:(hl + 1) * 64, gs], in0=av[0:64, :width],
                in1=rb[:, :width], op=MULT)

        def proj_unit(scn, n):
            ps = psp.tile([128, 512], F32, tag="sps")
            for kk in range(2):
                nc.tensor.matmul(
                    ps[:, :],
                    lhsT=wp_sb[:, kk, n * 128:(n + 1) * 128],
                    rhs=attT[kk][:, scn * 512:(scn + 1) * 512],
                    start=(kk == 0), stop=(kk == 1))
            osb = opool.tile([128, 512], BF16, tag="osb", name=f"osb{scn}_{n}")
            nc.vector.tensor_copy(out=osb[:, :], in_=ps[:, :])
            nc.sync.dma_start(
                out=outT.ap()[n * 128:(n + 1) * 128, scn * 512:(scn + 1) * 512],
                in_=osb[:, :])

        # ================= emission =================
        # pair-0 first half of qt/kt rides the input-DMA ramp
        for step in emit_qk_tile_steps(0, (0, 1)):
            step()

        # early scores (first half of sq space) woven with pair-0 (2,3) wave
        early = [blk + (hl,) for blk in
                 [(j, 0, j * 128, 1024) for j in range(8)]
                 for hl in range(2)]
        w1 = emit_qk_tile_steps(0, (2, 3))
        wi = 0
        for n, blk in enumerate(early):
            emit_scores_h(0, *blk)
            while wi * len(early) < (n + 1) * len(w1):
                w1[wi]()
                wi += 1
        while wi < len(w1):
            w1[wi]()
            wi += 1

        # late scores woven with v units and pair-1 qk waves
        late = [blk + (hl,) for blk in
                [(j, 1, max(j * 128, 1024), S) for j in range(SQT)]
                for hl in range(2)]
        w2 = ([lambda j=j: emit_v_unit(j) for j in range(SQT)] +
              emit_qk_tile_steps(1, (0, 1)) + emit_qk_tile_steps(1, (2, 3)))
        wi = 0
        for n, blk in enumerate(late):
            emit_scores_h(0, *blk)
            while wi * len(late) < (n + 1) * len(w2):
                w2[wi]()
                wi += 1
        while wi < len(w2):
            w2[wi]()
            wi += 1

        # pair-0 av woven with pair-1 scores; pair-1 av woven with proj
        sc1_A = [(j, 0, j * 128, 1024) for j in range(8)]
        sc1_Blo = [(j, 1, max(j * 128, 1024), S) for j in range(8)]
        sc1_Bhi = [(j, 1, j * 128, S) for j in range(8, SQT)]
        proj012 = [(sn, n) for sn in (0, 1, 2) for n in range(NX // 128)]
        proj3 = [(3, n) for n in range(NX // 128)]

        def sc1(b):
            return [lambda hl=hl: emit_scores_h(1, *b, hl) for hl in range(2)]

        def pj(u):
            return lambda: proj_unit(*u)

        def run_pair(p, weave, woff):
            total = sum((b + w) // 128 for (b, w) in
                        ((0, 512), (512, 512), (1024, 512), (1536, 512)))
            gstep, wi = 0, 0
            for (base, width) in ((0, 512), (512, 512), (1024, 512), (1536, 512)):
                njs = (base + width) // 128
                avs = [avp.tile([65, width], F32, tag="av",
                                name=f"av{p}{base}{hl}") for hl in range(2)]
                for j in range(njs):
                    while (wi < len(weave) and gstep >= woff and
                           wi * (total - woff) < (gstep - woff + 1) * len(weave)):
                        weave[wi]()
                        wi += 1
                    for hl in range(2):
                        emit_av(p, hl, base, width, j, avs[hl])
                    gstep += 1
                for hl in (1, 0):
                    emit_tail(p, hl, base, width, avs[hl])
            while wi < len(weave):
                weave[wi]()
                wi += 1

        run_pair(0, [f for b in sc1_A + sc1_Blo + sc1_Bhi for f in sc1(b)], 0)
        run_pair(1, [pj(u) for u in proj012], 4)

        # ---- projection (last column block) ----
        for u in proj3:
            proj_unit(*u)


def build_nc():
    import concourse.bass as bass
    import concourse.mybir as mybir
    import concourse.tile as tile
    from concourse import bacc
    dt = mybir.dt
    F32, BF16 = dt.float32, dt.bfloat16

    nc = bacc.Bacc("TRN2", target_bir_lowering=False, debug=False, num_devices=8)
    xT = nc.dram_tensor("xT", [NX, S], BF16, kind="ExternalInput")
    wqk = nc.dram_tensor("wqk", [NX, 2 * HPC * D], BF16, kind="ExternalInput")
    wv = nc.dram_tensor("wv", [NX, HPC * D], BF16, kind="ExternalInput")
    wp = nc.dram_tensor("wp", [HPC * D, NX], BF16, kind="ExternalInput")
    bq = nc.dram_tensor("bq", [HPC * D, 1], F32, kind="ExternalInput")
    bk = nc.dram_tensor("bk", [HPC * D, 1], F32, kind="ExternalInput")
    maskT = nc.dram_tensor("maskT", [128, 128], BF16, kind="ExternalInput")
    outT = nc.dram_tensor("outT", [NX, S], BF16, kind="ExternalOutput")
    tens = (xT, wqk, wv, wp, bq, bk, maskT, outT)

    with tile.TileContext(nc) as tc:
        _emit(nc, tc, bass, mybir, tens)
    nc.compile()
    return nc


def make_in_maps(x, w_attn, b_attn):
    bf = ml_dtypes.bfloat16
    maskT = np.triu(np.ones((128, 128), np.float32)).astype(bf)
    in_maps = []
    for core in range(8):
        b, g = divmod(core, 4)
        qs, ks, vs = 256 * g, NX + 256 * g, 2 * NX + 256 * g
        in_maps.append({
            "xT": np.ascontiguousarray(x[b].T).astype(bf),
            "wqk": np.ascontiguousarray(np.concatenate(
                [w_attn[:, qs:qs + 256], w_attn[:, ks:ks + 256]], axis=1)).astype(bf),
            "wv": np.ascontiguousarray(w_attn[:, vs:vs + 256]).astype(bf),
            "wp": None,  # filled by kernel() (needs w_proj)
            "bq": b_attn[qs:qs + 256].reshape(256, 1).astype(np.float32),
            "bk": b_attn[ks:ks + 256].reshape(256, 1).astype(np.float32),
            "maskT": maskT,
        })
    return in_maps


def kernel(**inputs):
    global _nc_cache
    x = np.asarray(inputs["x"], np.float32)
    w_attn = np.asarray(inputs["w_attn"], np.float32)
    b_attn = np.asarray(inputs["b_attn"], np.float32)
    w_proj = np.asarray(inputs["w_proj"], np.float32)
    b_proj = np.asarray(inputs["b_proj"], np.float32)

    bf = ml_dtypes.bfloat16
    in_maps = make_in_maps(x, w_attn, b_attn)
    for core in range(8):
        g = core % 4
        in_maps[core]["wp"] = np.ascontiguousarray(
            w_proj[256 * g:256 * (g + 1), :]).astype(bf)

    if _nc_cache is None:
        _nc_cache = build_nc()
    from concourse.bass_utils import run_bass_kernel_spmd
    res = run_bass_kernel_spmd(_nc_cache, in_maps, core_ids=list(range(8)))

    out = np.zeros((B, S, NX), np.float32)
    for core in range(8):
        out[core // 4] += res.results[core]["outT"].astype(np.float32).T
    bv = b_attn[2 * NX:3 * NX]
    out += (bv @ w_proj + b_proj)[None, None, :]
    return out
